# revision 16
# baseline (speedup 1.0000x reference)
"""Trainium2 Bass kernel for nn_BNet (hypergraph GNN message passing), 8 cores.

V2 strategy
-----------
All node/hyperedge intermediates live in degree-grouped, permuted order (the
final output is a (1,1) scalar after global pooling, so order is free):

- Nodes and hyperedges with degree>=1 are dealt round-robin per degree class
  across the 8 cores, tiled 128 at a time, one degree per tile. Segment
  reductions become fixed-width strided reduces; 1/deg becomes a
  compile-time scalar per tile.
- Phase A (hyperedge side): the host pre-gathers raw per-pin features
  pin36 = [h[node] | pin_feature] (fp16, hyperedge-grouped slots) — pure
  input relayout. The device strided-reduces the 36-wide raw sums per
  hyperedge, then one matmul per 128-hyperedge tile against
  rhs37 = [[W1;Wpin] | (W1;Wpin)@att2 ; d*(b1|b1@att2)] produces the full
  65-wide ef row (e_feat + v) directly. No xl table, no xl AllGather.
- u = xl@att1 per node is computed as h33 @ (W1b@att1) with one broadcast
  multiply + reduce over the host-supplied tile-major h layout.
- ef table is fp16; one AllGather shares it.
- Phase C: ONE batched indirect DMA per 16384-slot chunk gathers ef rows
  into padded 66-wide slots (padding keeps each row a separate descriptor;
  the HW merges contiguous out runs into single descriptors). Softmax
  attention + PNA run mostly in fp16 on the vector engine; Wpost via PE
  transpose + fp16 matmuls; pooled sums accumulate in PSUM; AllReduce;
  tiny fp32 MLP head.
"""

import numpy as np

import bass_rust
import concourse.bass as bass
import concourse.tile as tile
from concourse import mybir
from concourse.bass_utils import run_bass_kernel_spmd
from concourse.masks import make_identity
from concourse.vector_clock import ScopedClock

# ----------------------------------------------------------------- constants
N_NODES = 200000
N_HE = 100000
NNZ = 1000000
F_IN = 32  # 29 + 2 + 1
F36 = 36   # 32 node features + 4 pin features
C = 64
NCORES = 8
P = 128
W_EF = 65  # e_feat row: 64 dims + v
WP = 66    # padded gather slot stride (separates DMA descriptors)
K_CH = 128  # gather slots per partition per chunk
MAX_RUN = 8
SLOPE = 0.1
F32 = mybir.dt.float32
F16 = mybir.dt.float16
I32 = mybir.dt.int32
AX = mybir.AxisListType.X
AF = mybir.ActivationFunctionType


# ------------------------------------------------------- walrus workarounds
def _patched_drain_and_barrier(self, tick_clock, wait_clock):
    nc = self.nc
    assert self.sems is not None
    handles = list(self.sems.allocated().values())
    scratch = nc.sync.sem_inc(handles[0], 0) if handles else nc.sync.drain()
    wait_clock.add_sem_waits(scratch.ins, ScopedClock({None: tick_clock.global_clock}))
    waits = list(scratch.ins.sync_info.on_wait)
    scratch.ins.sync_info = bass_rust.SyncInfo(on_wait=[], on_update=[])
    by_name = {h.name: h for h in handles}
    for w in waits:
        nc.sync.wait_ge(by_name[w.ant_name], w.wait_value)
    nc.sync.drain()
    nc.all_engine_barrier()
    popped = nc._tile_sem_poison_stack.pop()
    assert popped is self._sem_poison
    nc.clear_and_free_semaphores(handles)
    nc.all_engine_barrier()


tile.TileContext._drain_and_barrier = _patched_drain_and_barrier

_WS_CTR = [0]


def _split_waits(nc):
    """This walrus build allows at most one sync-wait per instruction; hoist
    extras onto NoOps inserted just before, same engine."""
    for fn in nc.m.functions:
        for bb in fn.blocks:
            insts = list(bb.instructions)
            new = []
            for inst in insts:
                si = inst.sync_info
                if si is not None and len(si.on_wait) > 1:
                    waits = list(si.on_wait)
                    for w in waits[:-1]:
                        _WS_CTR[0] += 1
                        new.append(
                            mybir.InstNoOp(
                                name=f"waitsplit_{_WS_CTR[0]}",
                                engine=inst.engine,
                                sync_info=mybir.SyncInfo(on_wait=[w], on_update=[]),
                                bass_nofuse=True,
                            )
                        )
                    inst.sync_info = mybir.SyncInfo(
                        on_wait=[waits[-1]], on_update=list(si.on_update)
                    )
                new.append(inst)
            bb.instructions = new


# ----------------------------------------------------------- preprocessing
def _partition_by_degree(deg, ncores):
    """Deal ids with deg>=1 round-robin per degree class across cores.

    Returns (core, local_row, tiles, n_rows) where tiles is the common
    per-core tile list [(degree, base_row)] and n_rows includes one final
    all-dummy tile (guaranteed zero rows; last row is the gather sentinel).
    """
    n = len(deg)
    order = np.lexsort((np.arange(n), deg))
    order = order[deg[order] >= 1]
    d_sorted = deg[order].astype(np.int64)
    change = np.nonzero(np.diff(d_sorted))[0] + 1
    starts = np.r_[0, change]
    ends = np.r_[change, len(order)]
    rank = np.arange(len(order)) - np.repeat(starts, ends - starts)
    core_of = (rank % ncores).astype(np.int32)
    lrank = rank // ncores
    tiles = []
    local = np.zeros(len(order), np.int64)
    base = 0
    for s, e in zip(starts, ends):
        d = int(d_sorted[s])
        m = int(np.ceil((e - s) / ncores))
        t_d = int(np.ceil(m / P))
        idx = slice(s, e)
        local[idx] = base + lrank[idx]
        for t in range(t_d):
            tiles.append((d, base + t * P))
        base += t_d * P
    n_rows = base + P  # one extra all-dummy tile
    core = np.full(n, -1, np.int32)
    loc = np.full(n, -1, np.int64)
    core[order] = core_of
    loc[order] = local
    return core, loc, tiles, n_rows


def _pack_chunks(tiles):
    """Pack tiles into K_CH-slot chunks; a tile never crosses a chunk.
    Returns (placement [(chunk, col)], n_chunks, runs, used) with runs =
    [(chunk, col0, degree, [tile_indices])] capped at MAX_RUN tiles."""
    place = []
    chunk, cur = 0, 0
    used = {}
    for d, _ in tiles:
        if cur + d > K_CH:
            chunk += 1
            cur = 0
        place.append((chunk, cur))
        cur += d
        used[chunk] = cur
    n_chunks = chunk + 1
    runs = []
    i = 0
    while i < len(tiles):
        d = tiles[i][0]
        ch, col = place[i]
        j = i
        while (
            j + 1 < len(tiles)
            and tiles[j + 1][0] == d
            and place[j + 1][0] == ch
            and j + 1 - i + 1 <= MAX_RUN
        ):
            j += 1
        runs.append((ch, col, d, list(range(i, j + 1))))
        i = j + 1
    return place, n_chunks, runs, used


def _rank_within(seg_ids):
    """j-rank of each element within its segment (stable, segment-sorted)."""
    order = np.argsort(seg_ids, kind="stable")
    sorted_ids = seg_ids[order]
    change = np.nonzero(np.diff(sorted_ids))[0] + 1
    starts = np.r_[0, change]
    counts = np.diff(np.r_[starts, len(sorted_ids)])
    r = np.arange(len(sorted_ids)) - np.repeat(starts, counts)
    out = np.empty(len(seg_ids), np.int64)
    out[order] = r
    return out


def preprocess(inputs):
    x = np.asarray(inputs["x"])
    fake_pos = np.asarray(inputs["fake_pos"])
    edge_index = np.asarray(inputs["edge_index"])
    pin_feature = np.asarray(inputs["pin_feature"])
    macro_index = np.asarray(inputs["macro_index"])
    node_idx = edge_index[0].astype(np.int64)
    he_idx = edge_index[1].astype(np.int64)

    deg_n = np.bincount(node_idx, minlength=N_NODES)
    deg_e = np.bincount(he_idx, minlength=N_HE)

    core_n, loc_n, tiles_n, NRN = _partition_by_degree(deg_n, NCORES)
    core_e, loc_e, tiles_e, NRE = _partition_by_degree(deg_e, NCORES)
    placeA, nchA, runsA, usedA = _pack_chunks(tiles_e)
    placeC, nchC, runsC, usedC = _pack_chunks(tiles_n)

    erow = core_e.astype(np.int64) * NRE + loc_e  # ef_full row per hyperedge
    sent_e = NRE - 1

    # --- per-pin placement, hyperedge-major (phase A) ---
    jA = _rank_within(he_idx)
    cA = core_e[he_idx]
    tA = loc_e[he_idx] // P
    pA = loc_e[he_idx] % P
    chA = np.array([pl[0] for pl in placeA], np.int64)[tA]
    colA = np.array([pl[1] for pl in placeA], np.int64)[tA] + jA

    # --- node-major (phase C) ---
    jC = _rank_within(node_idx)
    cC = core_n[node_idx]
    tC = loc_n[node_idx] // P
    pC = loc_n[node_idx] % P
    chC = np.array([pl[0] for pl in placeC], np.int64)[tC]
    colC = np.array([pl[1] for pl in placeC], np.int64)[tC] + jC

    # ismacro / macro multiplicity
    ismacro = np.zeros(N_NODES, np.float32)
    ismacro[macro_index] = 1.0
    mult = np.bincount(macro_index, minlength=N_NODES).astype(np.float32)

    TN = len(tiles_n)
    h_full = np.concatenate([x, fake_pos, ismacro[:, None]], 1)  # (N, 32)
    pin36_all = np.concatenate(
        [h_full[node_idx], pin_feature], 1
    ).astype(np.float16)  # (NNZ, 36)

    per_core = []
    for c in range(NCORES):
        # phase A raw pin features, hyperedge-grouped slots
        pin36 = np.zeros((nchA, P, K_CH, F36), np.float16)
        m = cA == c
        pin36[chA[m], pA[m], colA[m]] = pin36_all[m]

        # phase C gather index
        cidx = np.full((nchC, P, K_CH), sent_e, np.int32)
        m2 = cC == c
        cidx[chC[m2], pC[m2], colC[m2]] = erow[he_idx[m2]].astype(np.int32)

        # tile-major node features for u ([h|1], 33 per node)
        sel = core_n == c
        hP = np.zeros((P, TN, 33), np.float16)
        nl = loc_n[sel]
        hP[nl % P, nl // P, :F_IN] = h_full[sel].astype(np.float16)
        hP[nl % P, nl // P, F_IN] = 1.0

        # pooling weights [128, TN*2]
        wpool = np.zeros((P, TN * 2), np.float32)
        wpool[nl % P, (nl // P) * 2] = mult[sel]
        wpool[nl % P, (nl // P) * 2 + 1] = 1.0

        per_core.append(
            dict(
                pin36=pin36.reshape(nchA, P, K_CH * F36),
                cidx=cidx,
                hP=hP.reshape(P, TN * 33),
                wpool=wpool,
            )
        )

    return dict(
        per_core=per_core,
        tiles_n=tiles_n,
        tiles_e=tiles_e,
        runsA=runsA,
        runsC=runsC,
        usedA=usedA,
        usedC=usedC,
        nchA=nchA,
        nchC=nchC,
        NRN=NRN,
        NRE=NRE,
        core_n=core_n,
        loc_n=loc_n,
        core_e=core_e,
        loc_e=loc_e,
    )


def _const_col_factory(nc, pool):
    cache = {}

    def cc(val, parts=P):
        key = (float(val), parts)
        if key not in cache:
            t = pool.tile([parts, 1], F32, tag=f"cc{len(cache)}")
            nc.vector.memset(t[:], float(val))
            cache[key] = t
        return cache[key][:]

    return cc


# ----------------------------------------------------------- device program
def build_program(prep):
    NRE = prep["NRE"]
    nchA, nchC = prep["nchA"], prep["nchC"]
    runsA, runsC = prep["runsA"], prep["runsC"]
    TN = len(prep["tiles_n"])
    TH = len(prep["tiles_e"])
    core_ids = list(range(NCORES))

    nc = bass.Bass("TRN2", target_bir_lowering=False, debug=False, num_devices=NCORES)

    # inputs
    pin36_in = nc.declare_dram_parameter("pin36", [nchA, P, K_CH * F36], F16, isOutput=False)
    hP_in = nc.declare_dram_parameter("hP", [P, TN * 33], F16, isOutput=False)
    cidx_in = nc.declare_dram_parameter("cidx", [nchC, P, K_CH], I32, isOutput=False)
    wpool_in = nc.declare_dram_parameter("wpool", [P, TN * 2], F32, isOutput=False)
    Wcat_in = nc.declare_dram_parameter("Wcat", [F36, C], F32, isOutput=False)
    WcatT_in = nc.declare_dram_parameter("WcatT", [C, F36], F32, isOutput=False)
    W1bT_in = nc.declare_dram_parameter("W1bT", [C, 33], F32, isOutput=False)
    b1r_in = nc.declare_dram_parameter("b1r", [1, C], F32, isOutput=False)
    b1c_in = nc.declare_dram_parameter("b1c", [C, 1], F32, isOutput=False)
    att1_in = nc.declare_dram_parameter("att1", [C, 1], F32, isOutput=False)
    att2_in = nc.declare_dram_parameter("att2", [C, 1], F32, isOutput=False)
    wpostA_in = nc.declare_dram_parameter("wpostA", [2 * C, C], F32, isOutput=False)
    wpostB_in = nc.declare_dram_parameter("wpostB", [2 * C, C], F32, isOutput=False)
    bpost_in = nc.declare_dram_parameter("bpost", [1, C], F32, isOutput=False)
    Wm1_in = nc.declare_dram_parameter("Wm1", [2 * C, C], F32, isOutput=False)
    bm1_in = nc.declare_dram_parameter("bm1", [1, C], F32, isOutput=False)
    Wm2_in = nc.declare_dram_parameter("Wm2", [C, C // 2], F32, isOutput=False)
    bm2_in = nc.declare_dram_parameter("bm2", [1, C // 2], F32, isOutput=False)
    Wm3_in = nc.declare_dram_parameter("Wm3", [C // 2, 1], F32, isOutput=False)
    bm3_in = nc.declare_dram_parameter("bm3", [1, 1], F32, isOutput=False)
    z_out = nc.declare_dram_parameter("z", [1, 1], F32, isOutput=True)

    # internal DRAM
    ef_shard = nc.dram_tensor("ef_shard", [NRE, W_EF], F16)
    ef_full = nc.dram_tensor("ef_full", [NCORES * NRE, W_EF], F16, addr_space="Shared")
    ar_in = nc.dram_tensor("ar_in", [2, C], F32)
    ar_out = nc.dram_tensor("ar_out", [2, C], F32, addr_space="Shared")

    u_sb = nc.alloc_sbuf_tensor("u_sb", [P, TN], F16)  # persistent u columns

    # ---------------- context 1: weight fold + u + phase A -----------------
    with tile.TileContext(nc) as tc:
        with (
            tc.tile_pool(name="cpool", bufs=1) as cpool,
            tc.tile_pool(name="apin", bufs=3) as apin,
            tc.tile_pool(name="arun", bufs=3) as arun,
        ):
            ident = cpool.tile([P, P], F32)
            make_identity(nc, ident[:])
            ccA = _const_col_factory(nc, cpool)

            # -- weight fold (own PSUM scope, freed before the chunk loop) --
            wcat = cpool.tile([F36, C], F32)
            nc.sync.dma_start(out=wcat[:], in_=Wcat_in[:, :])
            wcatT = cpool.tile([C, F36], F32)
            nc.sync.dma_start(out=wcatT[:], in_=WcatT_in[:, :])
            w1bT = cpool.tile([C, 33], F32)
            nc.sync.dma_start(out=w1bT[:], in_=W1bT_in[:, :])
            b1r = cpool.tile([1, C], F32)
            nc.sync.dma_start(out=b1r[:], in_=b1r_in[:, :])
            b1c = cpool.tile([C, 1], F32)
            nc.sync.dma_start(out=b1c[:], in_=b1c_in[:, :])
            att1 = cpool.tile([C, 1], F32)
            nc.sync.dma_start(out=att1[:], in_=att1_in[:, :])
            att2 = cpool.tile([C, 1], F32)
            nc.sync.dma_start(out=att2[:], in_=att2_in[:, :])
            hP_t = cpool.tile([P, TN * 33], F16)
            nc.sync.dma_start(out=hP_t[:], in_=hP_in[:, :])
            rhs36 = cpool.tile([F36, W_EF], F32)
            b65row = cpool.tile([1, W_EF], F32)
            ones1_f = cpool.tile([1, P], F32)
            nc.vector.memset(ones1_f[:], 1.0)
            w33T_sb = cpool.tile([1, 33], F32)
            w33r = cpool.tile([P, 33], F32)

            with tc.tile_pool(name="prepsum", bufs=1, space="PSUM") as prepsum:
                nc.scalar.activation(rhs36[:, :C], wcat[:], AF.Copy)
                psA = prepsum.tile([F36, 1], F32, space="PSUM", tag="psA")
                nc.tensor.matmul(psA[:], lhsT=wcatT[:], rhs=att2[:], start=True, stop=True)
                nc.scalar.activation(rhs36[:, C : C + 1], psA[:], AF.Copy)
                nc.scalar.activation(b65row[:, :C], b1r[:], AF.Copy)
                psB = prepsum.tile([1, 1], F32, space="PSUM", tag="psB")
                nc.tensor.matmul(psB[:], lhsT=b1c[:], rhs=att2[:], start=True, stop=True)
                nc.scalar.activation(b65row[:, C : C + 1], psB[:], AF.Copy)

                psW = prepsum.tile([1, 33], F32, space="PSUM", tag="psW")
                nc.tensor.matmul(psW[:], lhsT=att1[:], rhs=w1bT[:], start=True, stop=True)
                nc.vector.tensor_copy(w33T_sb[:], psW[:])
                psR = prepsum.tile([P, 33], F32, space="PSUM", tag="psR")
                nc.tensor.matmul(psR[:], lhsT=ones1_f[:], rhs=w33T_sb[:], start=True, stop=True)
                nc.scalar.activation(w33r[:], psR[:], AF.Copy)

            # -- u = sum_f h33 * w33 per node (tile-major layout) --
            tmp33 = cpool.tile([P, TN * 33], F32)
            nc.vector.tensor_tensor(
                out=tmp33[:].rearrange("p (t f) -> p t f", f=33),
                in0=hP_t[:].rearrange("p (t f) -> p t f", f=33),
                in1=w33r[:, None, :].to_broadcast([P, TN, 33]),
                op=mybir.AluOpType.mult,
            )
            with nc.allow_low_precision(reason="u is fp16 attention bias; 5e-4 ok"):
                nc.vector.reduce_sum(
                    u_sb[:, :TN],
                    tmp33[:].rearrange("p (t f) -> p t f", f=33),
                    axis=AX,
                )

            # zero the sentinel tile of ef_shard
            zt = cpool.tile([P, W_EF], F16)
            nc.vector.memset(zt[:], 0.0)
            nc.sync.dma_start(out=ef_shard[TH * P : (TH + 1) * P, :], in_=zt[:])

            run_by_chunkA = {}
            for r in runsA:
                run_by_chunkA.setdefault(r[0], []).append(r)

            with tc.tile_pool(name="atps", bufs=3, space="PSUM") as atps:
                for ch in range(nchA):
                    pint = apin.tile([P, K_CH * F36], F16)
                    nc.sync.dma_start(out=pint[:], in_=pin36_in[ch])
                    for (_, col, d, tl) in run_by_chunkA.get(ch, []):
                        T = len(tl)
                        t0 = tl[0]
                        rawsum = arun.tile([P, MAX_RUN * F36], F32, tag="rs")
                        nc.vector.reduce_sum(
                            rawsum[:, : T * F36].rearrange("p (t f) -> p t f", t=T),
                            pint[:, col * F36 : (col + T * d) * F36].rearrange(
                                "p (t j f) -> p t f j", t=T, j=d, f=F36
                            ),
                            axis=AX,
                        )
                        efb = arun.tile([P, MAX_RUN * W_EF], F16, tag="efb")
                        drow = arun.tile([1, P], F32, tag="dr")
                        nc.vector.memset(drow[:], float(d))
                        for ti in range(T):
                            trp = atps.tile([P, P], F32, space="PSUM", tag="trp")
                            nc.tensor.transpose(
                                out=trp[:F36, :],
                                in_=rawsum[:, ti * F36 : (ti + 1) * F36],
                                identity=ident[:],
                            )
                            lhsT36 = arun.tile([F36, P], F32, tag="lh")
                            nc.scalar.activation(lhsT36[:], trp[:F36, :], AF.Copy)
                            efps = atps.tile([P, W_EF], F32, space="PSUM", tag="efps")
                            nc.tensor.matmul(
                                efps[:], lhsT=lhsT36[:], rhs=rhs36[:], start=True, stop=False
                            )
                            nc.tensor.matmul(
                                efps[:], lhsT=drow[:], rhs=b65row[:], start=False, stop=True
                            )
                            nc.scalar.activation(
                                efb[:, ti * W_EF : (ti + 1) * W_EF],
                                efps[:],
                                AF.Copy,
                                scale=ccA(1.0 / d),
                            )
                        nc.sync.dma_start(
                            out=ef_shard[t0 * P : (t0 + T) * P, :].rearrange(
                                "(t p) w -> p t w", p=P
                            ),
                            in_=efb[:, : T * W_EF].rearrange("p (t w) -> p t w", t=T),
                        )

    # AllGather ef
    with (
        nc.semaphore("ag2_sem") as ag2_sem,
        nc.Block() as blk2,
    ):

        @blk2.gpsimd
        def _(g):
            g.collective_compute(
                "AllGather",
                mybir.AluOpType.bypass,
                replica_groups=[core_ids],
                ins=[ef_shard[:, :]],
                outs=[ef_full[:, :]],
            ).then_inc(ag2_sem, 1)
            g.wait_ge(ag2_sem, 1)

    # ---------------- context 3: phase C (attention + PNA + pooling) -------
    with tile.TileContext(nc) as tc:
        with (
            tc.tile_pool(name="ccons", bufs=1) as ccons,
            tc.tile_pool(name="cidxp", bufs=3) as cidxp,
            tc.tile_pool(name="cstg", bufs=4) as cstg,
            tc.tile_pool(name="cmsg", bufs=2) as cmsg,
            tc.tile_pool(name="csml", bufs=3) as csml,
            tc.tile_pool(name="cpost", bufs=2) as cpost,
            tc.tile_pool(name="chx", bufs=3) as chx,
            tc.tile_pool(name="cppsum", bufs=1, space="PSUM") as cppsum,
            tc.tile_pool(name="ctpsum", bufs=2, space="PSUM") as ctpsum,
        ):
            identC = ccons.tile([P, P], F16)
            make_identity(nc, identC[:])
            wpost_a = ccons.tile([P, C], F32)
            nc.sync.dma_start(out=wpost_a[:], in_=wpostA_in[:, :])
            wpost_b = ccons.tile([P, C], F32)
            nc.sync.dma_start(out=wpost_b[:], in_=wpostB_in[:, :])
            bpost = ccons.tile([1, C], F32)
            nc.sync.dma_start(out=bpost[:], in_=bpost_in[:, :])
            ones1h = ccons.tile([1, P], F32)
            nc.vector.memset(ones1h[:], 1.0)
            wpool = ccons.tile([P, TN * 2], F32)
            nc.sync.dma_start(out=wpool[:], in_=wpool_in[:, :])
            pool_ps = cppsum.tile([2, C], F32, space="PSUM")
            ccC = _const_col_factory(nc, ccons)

            run_by_chunkC = {}
            for r in runsC:
                run_by_chunkC.setdefault(r[0], []).append(r)

            first_mm = [True]
            n_tiles_done = [0]
            for ch in range(nchC):
                it = cidxp.tile([P, K_CH], I32)
                nc.sync.dma_start(out=it[:], in_=cidx_in[ch])
                for (_, col, d, tl) in run_by_chunkC.get(ch, []):
                    T = len(tl)
                    F = T * d
                    t0 = tl[0]
                    # per-run staging tile: consumers of run r overlap the
                    # gathers of run r+1 instead of waiting out the chunk
                    stg = cstg.tile([P, K_CH * W_EF], F16, tag="stg")
                    for j in range(F):
                        nc.gpsimd.indirect_dma_start(
                            out=stg[:, j * W_EF : (j + 1) * W_EF],
                            out_offset=None,
                            in_=ef_full[:, :],
                            in_offset=bass.IndirectOffsetOnAxis(
                                ap=it[:, col + j : col + j + 1], axis=0
                            ),
                        )
                    s3 = stg[:, : F * W_EF].rearrange(
                        "p (s w) -> p s w", s=F
                    )
                    ef_ap = s3[:, :, :C]
                    v_sc = s3[:, :, C]

                    # a = lrelu(u + v) ; ex = exp(a)
                    asl = csml.tile([P, K_CH], F16, tag="asl")
                    nc.vector.tensor_tensor(
                        out=asl[:, :F].rearrange("p (t j) -> p t j", t=T),
                        in0=u_sb[:, t0 : t0 + T, None].to_broadcast([P, T, d]),
                        in1=v_sc.rearrange("p (t j) -> p t j", t=T),
                        op=mybir.AluOpType.add,
                    )
                    a2 = csml.tile([P, K_CH], F16, tag="a2")
                    nc.vector.tensor_scalar_mul(a2[:, :F], asl[:, :F], SLOPE)
                    nc.vector.tensor_tensor(
                        out=asl[:, :F],
                        in0=asl[:, :F],
                        in1=a2[:, :F],
                        op=mybir.AluOpType.max,
                    )
                    ex = csml.tile([P, K_CH], F32, tag="ex")
                    nc.scalar.activation(ex[:, :F], asl[:, :F], AF.Exp)
                    den = csml.tile([P, MAX_RUN], F32, tag="den")
                    nc.vector.reduce_sum(
                        den[:, :T],
                        ex[:, :F].rearrange("p (t j) -> p t j", t=T),
                        axis=AX,
                    )
                    nc.vector.reciprocal(den[:, :T], den[:, :T])
                    alpha = csml.tile([P, K_CH], F16, tag="alpha")
                    nc.vector.tensor_tensor(
                        out=alpha[:, :F].rearrange("p (t j) -> p t j", t=T),
                        in0=ex[:, :F].rearrange("p (t j) -> p t j", t=T),
                        in1=den[:, :T, None].to_broadcast([P, T, d]),
                        op=mybir.AluOpType.mult,
                    )
                    # msg = alpha * e_g
                    msg = cmsg.tile([P, K_CH * C], F16, tag="msg")
                    nc.vector.tensor_tensor(
                        out=msg[:, : F * C].rearrange("p (s d) -> p s d", s=F),
                        in0=ef_ap,
                        in1=alpha[:, :F, None].to_broadcast([P, F, C]),
                        op=mybir.AluOpType.mult,
                    )
                    msq = cmsg.tile([P, K_CH * C], F16, tag="msq")
                    nc.scalar.activation(msq[:, : F * C], msg[:, : F * C], AF.Square)
                    msg3 = msg[:, : F * C].rearrange(
                        "p (t j d) -> p t d j", t=T, j=d, d=C
                    )
                    msq3 = msq[:, : F * C].rearrange(
                        "p (t j d) -> p t d j", t=T, j=d, d=C
                    )
                    # pna = [mean | mx | mn | std] per tile, 256 wide
                    post = cpost.tile([P, MAX_RUN * 4 * C], F16, tag="post")
                    post3 = post[:, : T * 4 * C].rearrange(
                        "p (t q d) -> p t q d", t=T, q=4
                    )
                    nc.vector.reduce_max(post3[:, :, 1], msg3, axis=AX)
                    nc.vector.tensor_reduce(
                        post3[:, :, 2], msg3, op=mybir.AluOpType.min, axis=AX
                    )
                    sm = cpost.tile([P, MAX_RUN * C], F32, tag="sm")
                    nc.vector.reduce_sum(
                        sm[:, : T * C].rearrange("p (t d) -> p t d", t=T), msg3, axis=AX
                    )
                    nc.scalar.activation(
                        post3[:, :, 0],
                        sm[:, : T * C].rearrange("p (t d) -> p t d", t=T),
                        AF.Copy,
                        scale=ccC(1.0 / d),
                    )
                    sq = cpost.tile([P, MAX_RUN * C], F32, tag="sq")
                    nc.vector.reduce_sum(
                        sq[:, : T * C].rearrange("p (t d) -> p t d", t=T), msq3, axis=AX
                    )
                    m2 = cpost.tile([P, MAX_RUN * C], F32, tag="m2")
                    nc.vector.tensor_tensor(
                        out=m2[:, : T * C].rearrange("p (t d) -> p t d", t=T),
                        in0=post3[:, :, 0],
                        in1=post3[:, :, 0],
                        op=mybir.AluOpType.mult,
                    )
                    nc.vector.tensor_scalar(
                        out=sq[:, : T * C],
                        in0=sq[:, : T * C],
                        scalar1=1.0 / d,
                        scalar2=None,
                        op0=mybir.AluOpType.mult,
                    )
                    nc.vector.tensor_tensor(
                        out=sq[:, : T * C],
                        in0=sq[:, : T * C],
                        in1=m2[:, : T * C],
                        op=mybir.AluOpType.subtract,
                    )
                    nc.vector.tensor_scalar_max(sq[:, : T * C], sq[:, : T * C], 0.0)
                    nc.scalar.activation(
                        post3[:, :, 3],
                        sq[:, : T * C].rearrange("p (t d) -> p t d", t=T),
                        AF.Sqrt,
                        bias=ccC(1e-12),
                    )
                    # hx = lrelu(pna @ Wpost + bpost) per tile, then pool matmul
                    for ti in range(T):
                        t = t0 + ti
                        pn = post[:, ti * 4 * C : (ti + 1) * 4 * C]
                        pT_ps = ctpsum.tile([P, P], F16, space="PSUM", tag="pT")
                        nc.tensor.transpose(
                            out=pT_ps[:], in_=pn[:, :P], identity=identC[:]
                        )
                        pT = chx.tile([P, 2 * P], F32, tag="pT_sb")
                        nc.scalar.activation(pT[:, :P], pT_ps[:], AF.Copy)
                        pT_ps2 = ctpsum.tile([P, P], F16, space="PSUM", tag="pT2")
                        nc.tensor.transpose(
                            out=pT_ps2[:], in_=pn[:, P:], identity=identC[:]
                        )
                        nc.scalar.activation(pT[:, P:], pT_ps2[:], AF.Copy)
                        hx_ps = ctpsum.tile([P, C], F32, space="PSUM", tag="hx")
                        nc.tensor.matmul(
                            hx_ps[:], lhsT=pT[:, :P], rhs=wpost_a[:],
                            start=True, stop=False,
                        )
                        nc.tensor.matmul(
                            hx_ps[:], lhsT=pT[:, P:], rhs=wpost_b[:],
                            start=False, stop=False,
                        )
                        nc.tensor.matmul(
                            hx_ps[:],
                            lhsT=ones1h[:],
                            rhs=bpost[:],
                            start=False,
                            stop=True,
                        )
                        hx0 = chx.tile([P, C], F32, tag="hx0_sb")
                        nc.scalar.activation(hx0[:], hx_ps[:], AF.Copy)
                        hxm = chx.tile([P, C], F32, tag="hxm_sb")
                        nc.scalar.activation(hxm[:], hx_ps[:], AF.Copy, scale=ccC(SLOPE))
                        hx = chx.tile([P, C], F32, tag="hx_sb")
                        nc.vector.tensor_tensor(
                            out=hx[:], in0=hx0[:], in1=hxm[:],
                            op=mybir.AluOpType.max,
                        )
                        n_tiles_done[0] += 1
                        nc.tensor.matmul(
                            pool_ps[:],
                            lhsT=wpool[:, 2 * t : 2 * t + 2],
                            rhs=hx[:],
                            start=first_mm[0],
                            stop=(n_tiles_done[0] == TN),
                        )
                        first_mm[0] = False

            pool_sb = ccons.tile([2, C], F32)
            nc.vector.tensor_copy(pool_sb[:], pool_ps[:])
            nc.sync.dma_start(out=ar_in[:, :], in_=pool_sb[:])

    # AllReduce pooled partials
    with (
        nc.semaphore("ar_sem") as ar_sem,
        nc.Block() as blk3,
    ):

        @blk3.gpsimd
        def _(g):
            g.collective_compute(
                "AllReduce",
                mybir.AluOpType.add,
                replica_groups=[core_ids],
                ins=[ar_in[:, :]],
                outs=[ar_out[:, :]],
            ).then_inc(ar_sem, 1)
            g.wait_ge(ar_sem, 1)

    # ---------------- context 4: MLP head ---------------------------------
    with tile.TileContext(nc) as tc:
        with (
            tc.tile_pool(name="mpool", bufs=1) as mpool,
            tc.tile_pool(name="mpsum", bufs=1, space="PSUM") as mpsum,
        ):
            identM = mpool.tile([P, P], F32)
            make_identity(nc, identM[:])
            onesM = mpool.tile([1, 1], F32)
            nc.vector.memset(onesM[:], 1.0)
            ccM = _const_col_factory(nc, mpool)
            pool2 = mpool.tile([2, C], F32)
            nc.sync.dma_start(out=pool2[:], in_=ar_out[:, :])
            poolT_ps = mpsum.tile([P, P], F32, space="PSUM")
            nc.tensor.transpose(out=poolT_ps[:C, :2], in_=pool2[:], identity=identM[:2, :2])
            pooled = mpool.tile([P, 1], F32)
            nc.scalar.activation(
                pooled[:C, :], poolT_ps[:C, :1], AF.Copy, scale=ccM(1.0 / 512.0, C)
            )
            nc.scalar.activation(
                pooled[C:, :], poolT_ps[:C, 1:2], AF.Copy, scale=ccM(1.0 / N_NODES, C)
            )
            wm1 = mpool.tile([2 * C, C], F32)
            nc.sync.dma_start(out=wm1[:], in_=Wm1_in[:, :])
            bm1 = mpool.tile([1, C], F32)
            nc.sync.dma_start(out=bm1[:], in_=bm1_in[:, :])
            wm2 = mpool.tile([C, C // 2], F32)
            nc.sync.dma_start(out=wm2[:], in_=Wm2_in[:, :])
            bm2 = mpool.tile([1, C // 2], F32)
            nc.sync.dma_start(out=bm2[:], in_=bm2_in[:, :])
            wm3 = mpool.tile([C // 2, 1], F32)
            nc.sync.dma_start(out=wm3[:], in_=Wm3_in[:, :])
            bm3 = mpool.tile([1, 1], F32)
            nc.sync.dma_start(out=bm3[:], in_=bm3_in[:, :])

            def _lrelu_row(dst, src_ps, width):
                tmp = mpool.tile([1, width], F32, tag=f"lr{width}")
                nc.scalar.activation(tmp[:], src_ps[:], AF.Copy, scale=ccM(SLOPE, 1))
                nc.vector.tensor_tensor(
                    out=dst[:], in0=src_ps[:], in1=tmp[:], op=mybir.AluOpType.max
                )

            z1_ps = mpsum.tile([1, C], F32, space="PSUM")
            nc.tensor.matmul(z1_ps[:], lhsT=pooled[:], rhs=wm1[:], start=True, stop=False)
            nc.tensor.matmul(
                z1_ps[:], lhsT=onesM[:].to_broadcast([1, 1]), rhs=bm1[:],
                start=False, stop=True,
            )
            z1 = mpool.tile([1, C], F32)
            _lrelu_row(z1, z1_ps, C)
            z1T_ps = mpsum.tile([P, P], F32, space="PSUM")
            nc.tensor.transpose(out=z1T_ps[:C, :1], in_=z1[:], identity=identM[:1, :1])
            z1T = mpool.tile([C, 1], F32)
            nc.vector.tensor_copy(z1T[:], z1T_ps[:C, :1])
            z2_ps = mpsum.tile([1, C // 2], F32, space="PSUM")
            nc.tensor.matmul(z2_ps[:], lhsT=z1T[:], rhs=wm2[:], start=True, stop=False)
            nc.tensor.matmul(
                z2_ps[:], lhsT=onesM[:].to_broadcast([1, 1]), rhs=bm2[:],
                start=False, stop=True,
            )
            z2 = mpool.tile([1, C // 2], F32)
            _lrelu_row(z2, z2_ps, C // 2)
            z2T_ps = mpsum.tile([P, P], F32, space="PSUM")
            nc.tensor.transpose(out=z2T_ps[: C // 2, :1], in_=z2[:], identity=identM[:1, :1])
            z2T = mpool.tile([C // 2, 1], F32)
            nc.vector.tensor_copy(z2T[:], z2T_ps[: C // 2, :1])
            z3_ps = mpsum.tile([1, 1], F32, space="PSUM")
            nc.tensor.matmul(z3_ps[:], lhsT=z2T[:], rhs=wm3[:], start=True, stop=False)
            nc.tensor.matmul(
                z3_ps[:], lhsT=onesM[:].to_broadcast([1, 1]), rhs=bm3[:],
                start=False, stop=True,
            )
            z3 = mpool.tile([1, 1], F32)
            nc.vector.tensor_copy(z3[:], z3_ps[:])
            nc.sync.dma_start(out=z_out[:, :], in_=z3[:])

    _split_waits(nc)
    return nc


def make_in_maps(prep, inputs):
    W1 = np.asarray(inputs["W1"], np.float32)
    b1 = np.asarray(inputs["b1"], np.float32)
    Wpin = np.asarray(inputs["Wpin"], np.float32)
    att = np.asarray(inputs["att"], np.float32)
    Wpost = np.asarray(inputs["Wpost"], np.float32)
    Wcat = np.vstack([W1, Wpin]).astype(np.float32)  # (36, 64)
    W1b = np.vstack([W1, b1[None, :]]).astype(np.float32)  # (33, 64)
    in_maps = []
    for c in range(NCORES):
        pc = prep["per_core"][c]
        in_maps.append(
            dict(
                pin36=pc["pin36"],
                hP=pc["hP"],
                cidx=pc["cidx"],
                wpool=pc["wpool"],
                Wcat=Wcat,
                WcatT=Wcat.T.copy(),
                W1bT=W1b.T.copy(),
                b1r=b1[None, :].copy(),
                b1c=b1[:, None].copy(),
                att1=att[:C, None].copy(),
                att2=att[C:, None].copy(),
                wpostA=Wpost[: 2 * C].copy(),
                wpostB=Wpost[2 * C :].copy(),
                bpost=np.asarray(inputs["bpost"], np.float32)[None, :],
                Wm1=np.asarray(inputs["Wm1"], np.float32),
                bm1=np.asarray(inputs["bm1"], np.float32)[None, :],
                Wm2=np.asarray(inputs["Wm2"], np.float32),
                bm2=np.asarray(inputs["bm2"], np.float32)[None, :],
                Wm3=np.asarray(inputs["Wm3"], np.float32),
                bm3=np.asarray(inputs["bm3"], np.float32)[None, :],
            )
        )
    return in_maps


def _install_ntff_hook():
    """Register the NTFF profile hook trn_boot skips when antenv.axon_hooks is
    absent, so run_bass_kernel_spmd(trace=True) can report exec_time_ns."""
    import sys
    import types

    try:
        if "antenv.axon_hooks" not in sys.modules:
            import antenv

            mod = types.ModuleType("antenv.axon_hooks")
            holder = [None]
            mod.set_axon_ntff_profile_hook = lambda h: holder.__setitem__(0, h)
            mod.get_axon_ntff_profile_hook = lambda: holder[0]
            mod._holder = holder
            sys.modules["antenv.axon_hooks"] = mod
            antenv.axon_hooks = mod
        mod = sys.modules["antenv.axon_hooks"]
        if mod.get_axon_ntff_profile_hook() is None:
            from trn_agent_boot.trn_boot import _ntff_profile_via_ctypes

            mod.set_axon_ntff_profile_hook(
                _ntff_profile_via_ctypes("/opt/axon/libaxon_pjrt.so")
            )
        return mod.get_axon_ntff_profile_hook() is not None
    except Exception:
        return False


_LAST = {}


def kernel(**inputs):
    prep = preprocess(inputs)
    nc = build_program(prep)
    in_maps = make_in_maps(prep, inputs)
    trace_ok = _install_ntff_hook()
    try:
        res = run_bass_kernel_spmd(
            nc, in_maps, list(range(NCORES)), trace=trace_ok, trace_cores=[0]
        )
    except Exception:
        res = run_bass_kernel_spmd(nc, in_maps, list(range(NCORES)))
    _LAST["res"] = res
    return res.results[0]["z"].astype(np.float32)
